# revision 14
# baseline (speedup 1.0000x reference)
"""Tensor-parallel GQA attention block for 8 Trainium2 NeuronCores.

Sharding: 32 q-heads / 8 kv-heads split across 8 cores (4 q-heads + 1
kv-head each).  Each core projects q/k/v from the full x, applies RoPE,
runs causal attention for its heads, AllGathers head outputs at
(s-tile, head-pair) granularity, and computes a distinct 256-column
slice of the final wo projection (emitted d-major; host transposes).

Attention inner loop processes q-heads in PAIRS: the two heads' score
matmuls use disjoint PE row groups (partition bases 0/64) so the
hardware runs them concurrently; their exp is one batched ACT call over
a 2-bank [128, 1024] PSUM tile; their attn@v matmuls share the vaug
stationary.  Score matmul for chunk kc+1 is emitted before attn@v of
chunk kc so the PE never drains while ACT computes exp.  The softmax
denominator reciprocal runs on the DVE (reciprocal_approx_fast), so the
ACT engine's spline tables never leave the Exp set.  The gathered
og_all row order is [head-pair-half, core, head, dh]; the host permutes
woT rows to match.  No fp32 matmuls anywhere (keeps FWL eligible).
"""

import sys

sys.path.insert(0, "/opt/trn_rl_repo")

import numpy as np
import ml_dtypes
from collections import deque
from contextlib import ExitStack

import concourse.bass as bass
import concourse.tile as tile
from concourse.tile_rust import add_dep_helper
from concourse import bacc, mybir
from concourse.bass import ds
from concourse.bass_utils import run_bass_kernel_spmd

BF16 = ml_dtypes.bfloat16
F32 = mybir.dt.float32
BF = mybir.dt.bfloat16

N_CORES = 8
S = 2048          # sequence length
D = 2048          # model dim
DH = 64           # head dim
HPC = 4           # q heads per core
THETA = 10000.0
ST = 512          # s-tile (free dim) size
NT = S // ST      # 4 s-tiles
DK = D // 128     # 16 contraction chunks
OC = HPC * DH     # 256 head-output columns per core

_CACHE = {}
LAST_RESULT = None


def _build_program():
    nc = bacc.Bacc("TRN2", target_bir_lowering=False, debug=False,
                   num_devices=N_CORES)

    def din(name, shape, dt):
        return nc.dram_tensor(name, shape, dt, kind="ExternalInput")

    xT_d = din("xT", [D, S], BF)
    wq_d = din("wqT", [D, OC], BF)
    wkv_d = din("wkvT", [D, 128], BF)      # columns: [v(64), k(64)]
    wo_d = din("woT", [D, OC], BF)         # rows o (pair-major), cols d-slice
    cosq_d = din("cosq", [128, S], BF)
    sinq_d = din("sinq", [128, S], BF)
    cosk_d = din("cosk", [128, S], BF)    # k tables live in rows 64..127
    sink_d = din("sink", [128, S], BF)
    tri_d = din("tri", [128, 128], BF)
    msw_d = din("mswap", [128, 128], BF)
    idn_d = din("ident", [128, 128], BF)

    y_d = nc.dram_tensor("y", [OC, S], F32, kind="ExternalOutput")  # d-major
    og_p = [[nc.dram_tensor(f"og{i}_{jp}", [128, ST], BF) for jp in range(2)]
            for i in range(NT)]
    oga_p = [[nc.dram_tensor(f"og_all{i}_{jp}", [N_CORES * 128, ST], BF,
                             addr_space="Shared") for jp in range(2)]
             for i in range(NT)]

    with tile.TileContext(nc) as tc:
        with ExitStack() as ctx:
            cp = ctx.enter_context(tc.tile_pool(name="const", bufs=1))
            psc = ctx.enter_context(tc.tile_pool(name="psc", bufs=2, space="PSUM"))
            pavp = ctx.enter_context(tc.tile_pool(name="pavp", bufs=2, space="PSUM"))
            psm = ctx.enter_context(tc.tile_pool(name="psm", bufs=2, space="PSUM"))
            rawp = ctx.enter_context(tc.tile_pool(name="raw", bufs=3))
            tmpp = ctx.enter_context(tc.tile_pool(name="tmp", bufs=3))
            expp = ctx.enter_context(tc.tile_pool(name="expp", bufs=4))
            nrmp = ctx.enter_context(tc.tile_pool(name="nrm", bufs=3))

            scr_d = nc.dram_tensor("scratch", [128, 8], F32)

            # ---- PE warm-up: enough to flip the HAM activity monitor to
            # full clock while the first input DMAs stream in.
            wsrc = cp.tile([128, ST], BF)
            nc.vector.memset(wsrc[:], 0.25)
            pw = psm.tile([128, ST], F32, tag="mm")
            NWARM = 12
            for i in range(NWARM):
                nc.tensor.matmul(pw[:], wsrc[:, 0:128], wsrc[:],
                                 start=(i == 0), stop=(i == NWARM - 1),
                                 skip_group_check=True)
            wout = rawp.tile([128, 8], F32, tag="wout")
            nc.vector.tensor_copy(wout[:], pw[:, 0:8])
            nc.gpsimd.dma_start(scr_d.ap(), wout[:])

            # ---- resident tensors (load order = consumption order) ----
            wkv_sb = cp.tile([128, DK, 128], BF)
            nc.sync.dma_start(wkv_sb[:], wkv_d.ap().rearrange("(ko p) m -> p ko m", p=128))
            bigs = [cp.tile([128, DK, ST], BF, name=f"big{i}", tag=f"big{i}") for i in range(NT)]
            for g in range(4):
                nc.sync.dma_start(
                    bigs[0][:, ds(4 * g, 4), :],
                    xT_d[ds(512 * g, 512), ds(0, ST)].rearrange("(ko p) s -> p ko s", p=128))
            wq_sb = cp.tile([128, DK, OC], BF)
            nc.sync.dma_start(wq_sb[:], wq_d.ap().rearrange("(ko p) m -> p ko m", p=128))
            # rope/mask tables go on the ACT hwdge queue so they are not
            # serialized behind the bulk x/weight loads on the sync queue
            cosk = cp.tile([128, S], BF); nc.scalar.dma_start(cosk[:], cosk_d.ap())
            sink = cp.tile([128, S], BF); nc.scalar.dma_start(sink[:], sink_d.ap())
            cosq = cp.tile([128, S], BF); nc.scalar.dma_start(cosq[:], cosq_d.ap())
            sinq = cp.tile([128, S], BF); nc.scalar.dma_start(sinq[:], sinq_d.ap())
            tri = cp.tile([128, 128], BF); nc.scalar.dma_start(tri[:], tri_d.ap())
            msw = cp.tile([128, 128], BF); nc.scalar.dma_start(msw[:], msw_d.ap())
            idn = cp.tile([128, 128], BF); nc.scalar.dma_start(idn[:], idn_d.ap())
            for t_ in (1, 2, 3):
                for g in range(4):
                    nc.sync.dma_start(
                        bigs[t_][:, ds(4 * g, 4), :],
                        xT_d[ds(512 * g, 512), ds(t_ * ST, ST)].rearrange("(ko p) s -> p ko s", p=128))
            wo_sb = cp.tile([128, DK, OC], BF)
            nc.sync.dma_start(wo_sb[:], wo_d.ap().rearrange("(ko p) m -> p ko m", p=128))

            qT = cp.tile([128, 2, S], BF)
            kT2 = cp.tile([128, S], BF)
            vaug = cp.tile([128, DK, DH + 1], BF)
            nc.vector.memset(vaug[:, :, DH:DH + 1], 1.0)

            # ---- phase 1 (projections + RoPE) as a list of small thunks
            # woven into the attention stream so dense PE work fills the
            # gaps of the ACT-paced attention pipeline.
            state = {}

            def mk_proj_thunks(t, j):
                X = bigs[t]

                def accum(d0, d1, first):
                    def th():
                        if first:
                            state[(t, j)] = psm.tile([128, ST], F32, tag="mm", name="projps")
                        ps = state[(t, j)]
                        for d in range(d0, d1):
                            lhsT = (wq_sb[:, d, ds(128 * j, 128)] if j < 2
                                    else wkv_sb[:, d, :])
                            nc.tensor.matmul(ps[:], lhsT, X[:, d, ds(0, ST)],
                                             start=(d == 0), stop=(d == DK - 1),
                                             skip_group_check=True)
                    return th

                def post():
                    ps = state.pop((t, j))
                    sl = ds(t * ST, ST)
                    raw = rawp.tile([128, ST], BF, tag="raw")
                    nc.vector.tensor_copy(raw[:], ps[:])
                    ps2 = psm.tile([128, ST], F32, tag="mm")
                    nc.tensor.matmul(ps2[:], msw[:], raw[:], start=True, stop=True)
                    if j < 2:
                        t1 = tmpp.tile([128, ST], F32, tag="tmp")
                        nc.vector.tensor_mul(t1[:], ps[:], cosq[:, sl])
                        t2 = tmpp.tile([128, ST], F32, tag="tmp")
                        nc.vector.tensor_mul(t2[:], ps2[:], sinq[:, sl])
                        nc.vector.tensor_add(qT[:, j, sl], t1[:], t2[:])
                    else:
                        t1 = tmpp.tile([128, ST], F32, tag="tmp")
                        nc.vector.tensor_mul(t1[64:128], ps[64:128], cosk[64:128, sl])
                        t2 = tmpp.tile([128, ST], F32, tag="tmp")
                        nc.vector.tensor_mul(t2[64:128], ps2[64:128], sink[64:128, sl])
                        nc.vector.tensor_add(kT2[64:128, sl], t1[64:128], t2[64:128])
                        # duplicate rotated k at partition base 0 (row-pack pair)
                        psd = psm.tile([64, ST], F32, tag="mm", name="psd")
                        nc.tensor.matmul(psd[:], idn[64:128, 64:128], kT2[64:128, sl],
                                         start=True, stop=True)
                        nc.vector.tensor_copy(kT2[0:64, sl], psd[:])
                        # v into [sk, dh] layout via PE transpose
                        for j4 in range(4):
                            pv = psm.tile([128, DH], BF, tag="mm", name="pv")
                            nc.tensor.transpose(pv[:], raw[0:64, ds(128 * j4, 128)],
                                                idn[0:64, 0:64])
                            nc.vector.tensor_copy(vaug[:, 4 * t + j4, 0:DH], pv[:])

                return [accum(0, 4, True), accum(4, 8, False),
                        accum(8, 12, False), accum(12, 16, False), post]

            # ---- phase 3: output projection, computed d-major ([d, s]) so
            # the wo stationaries stream 512-wide; host transposes at the
            # end.  Split at the half-contraction point so the first half
            # can run as soon as the pair-0 AllGather of that s-tile lands.
            p3state = {}

            def p3_mm(qt, dcol, o0, o1):
                X = bigs[qt]

                def th():
                    if o0 == 0:
                        p3state[(qt, dcol)] = psm.tile([128, ST], F32, tag="mm", name="p3py")
                    py = p3state[(qt, dcol)]
                    first = None
                    for oc in range(o0, o1):
                        mi = nc.tensor.matmul(py[:], wo_sb[:, oc, ds(128 * dcol, 128)],
                                              X[:, oc, :],
                                              start=(oc == 0), stop=(oc == DK - 1),
                                              skip_group_check=True)
                        if first is None:
                            first = mi
                    if o1 == DK:
                        py = p3state.pop((qt, dcol))
                        yts = tmpp.tile([128, ST], F32, tag="yts")
                        nc.scalar.copy(yts[:], py[:])
                        nc.scalar.dma_start(y_d[ds(128 * dcol, 128), ds(qt * ST, ST)],
                                            yts[:])
                    return first
                return th

            def mk_p3A(qt):
                return [p3_mm(qt, 0, 0, 4), p3_mm(qt, 0, 4, 8),
                        p3_mm(qt, 1, 0, 4), p3_mm(qt, 1, 4, 8)]

            def mk_p3B(qt):
                return [p3_mm(qt, 0, 8, 12), p3_mm(qt, 0, 12, 16),
                        p3_mm(qt, 1, 8, 12), p3_mm(qt, 1, 12, 16)]

            # ---- attention ----
            EXP = mybir.ActivationFunctionType.Exp

            def emit_norm(pav, t, jp, hh):
                # copy numerator+denominator out of PSUM right away (frees
                # the pav bank for the next pair), then normalize from SBUF.
                # the den row is copied to base 0 separately: custom-DVE ops
                # mis-handle partition-base offsets.
                oc = nrmp.tile([64, ST], F32, tag="oc")
                nc.vector.tensor_copy(oc[:], pav[0:DH, :])
                den = nrmp.tile([1, ST], F32, tag="den")
                nc.vector.tensor_copy(den[:], pav[DH:DH + 1, :])
                rec = nrmp.tile([1, ST], F32, tag="rec")
                nc.vector.reciprocal_approx_fast(rec[:], den[:])
                rep = nrmp.tile([64, ST], F32, tag="rep")
                nc.gpsimd.partition_broadcast(rep[:], rec[:])
                on = nrmp.tile([64, ST], BF, tag="on")
                nc.vector.tensor_mul(on[:], oc[:], rep[:])
                return nc.gpsimd.dma_start(og_p[t][jp][ds(DH * hh, DH), :], on[:])

            def emit_spair(t, jp, kc):
                sl = ds(t * ST, ST)
                ps = psc.tile([128, 1024], F32, tag="sc")
                nc.tensor.matmul(ps[:, 0:512], kT2[0:64, ds(128 * kc, 128)],
                                 qT[0:64, jp, sl], start=True, stop=True)
                nc.tensor.matmul(ps[:, 512:1024], kT2[64:128, ds(128 * kc, 128)],
                                 qT[64:128, jp, sl], start=True, stop=True)
                return ps

            # prologue: proj(t0) runs inline; kv first (longest dep chain
            # to the first score matmul).
            for th in (mk_proj_thunks(0, 2) + mk_proj_thunks(0, 0)
                       + mk_proj_thunks(0, 1)):
                th()

            wq = deque()
            for t in range(NT):
                if t + 1 < NT:
                    for j in (2, 0, 1):
                        wq.extend(mk_proj_thunks(t + 1, j))
                if t == 3:
                    for qt in range(2):
                        wq.extend(mk_p3A(qt))
                        wq.extend(mk_p3B(qt))
                    wq.extend(mk_p3A(2))
                nkc = 4 * t + 4
                nsteps = 2 * nkc
                # distribute woven work over the first ~3/4 of the block for
                # t<3 (so next tile's projections land before its block
                # starts); spread fully at t=3 (tail work arrives late).
                npace = nsteps if t == 3 else max(1, (3 * nsteps) // 4)
                thunks = list(wq)
                wq.clear()
                L = len(thunks)
                step = 0
                for jp in range(2):
                    pav0 = pavp.tile([128, ST], F32, tag="pav")
                    pav1 = pavp.tile([128, ST], F32, tag="pav")
                    ps_cur = emit_spair(t, jp, 0)
                    for kc in range(nkc):
                        ps_nxt = emit_spair(t, jp, kc + 1) if kc + 1 < nkc else None
                        # weave dense PE work into the exp wait
                        if step < npace:
                            lo = (step * L) // npace
                            hi = ((step + 1) * L) // npace
                            for th in thunks[lo:hi]:
                                th()
                        step += 1
                        et = expp.tile([128, 1024], BF, tag="exp")
                        nc.scalar.activation(et[:], ps_cur[:], EXP)
                        c = kc - 4 * t
                        if c >= 0:
                            for o in (0, 512):
                                if c > 0:
                                    nc.gpsimd.memset(et[:, o:o + 128 * c], 0.0)
                                nc.gpsimd.tensor_mul(
                                    et[:, ds(o + 128 * c, 128)],
                                    et[:, ds(o + 128 * c, 128)], tri[:])
                        nc.tensor.matmul(pav0[0:DH + 1, :], vaug[:, kc, :],
                                         et[:, 0:512],
                                         start=(kc == 0), stop=(kc == nkc - 1),
                                         skip_group_check=True)
                        nc.tensor.matmul(pav1[0:DH + 1, :], vaug[:, kc, :],
                                         et[:, 512:1024],
                                         start=(kc == 0), stop=(kc == nkc - 1),
                                         skip_group_check=True)
                        ps_cur = ps_nxt
                    og_i0 = emit_norm(pav0, t, jp, 0)
                    og_i1 = emit_norm(pav1, t, jp, 1)
                    nc.gpsimd.collective_compute(
                        "AllGather", mybir.AluOpType.bypass,
                        replica_groups=[list(range(N_CORES))],
                        ins=[og_p[t][jp].ap()], outs=[oga_p[t][jp].ap()])
                    nc.sync.dma_start(
                        bigs[t][:, ds(8 * jp, 8), :],
                        oga_p[t][jp].ap().rearrange("(ko p) s -> p ko s", p=128))

            # endgame: held-back wo work, gated on the first og write of the
            # last pair so it executes DURING the final AllGather instead of
            # draining early, keeping the PE busy through the collective.
            gate_mm = None
            for th in mk_p3B(2) + mk_p3A(3) + mk_p3B(3):
                mi = th()
                if gate_mm is None and mi is not None:
                    gate_mm = mi
            add_dep_helper(og_i0.ins, gate_mm.ins, sync=True,
                           reason="hold endgame wo work until last og write")

    nc.compile()
    return nc


def _host_prep(x, wq, wk, wv, wo, pos):
    x2 = np.ascontiguousarray(np.asarray(x).reshape(S, D))
    xT = np.ascontiguousarray(x2.T).astype(BF16)

    posf = np.asarray(pos).astype(np.float32)
    fr = (1.0 / (np.float32(THETA) **
                 (np.arange(0, DH, 2, dtype=np.float32) / np.float32(DH))))
    pf = posf[:, None] * fr[None, :]              # [S, 32] f32
    cos = np.cos(pf).astype(np.float32)
    sin = np.sin(pf).astype(np.float32)
    pidx = np.arange(128)
    fi = (pidx % DH) // 2
    sign = np.where(pidx % 2 == 0, np.float32(-1.0), np.float32(1.0))
    cosq = np.ascontiguousarray(cos[:, fi].T)                  # [128, S]
    sinq = np.ascontiguousarray((sin[:, fi] * sign[None, :]).T)
    kscale = np.float32(1.0 / np.sqrt(DH))
    cosk = np.zeros((128, S), np.float32)
    sink = np.zeros((128, S), np.float32)
    cosk[64:128] = cosq[0:64] * kscale
    sink[64:128] = sinq[0:64] * kscale
    cosq = cosq.astype(BF16); sinq = sinq.astype(BF16)
    cosk = cosk.astype(BF16); sink = sink.astype(BF16)

    tri = np.triu(np.ones((128, 128), np.float32)).astype(BF16)
    msw = np.zeros((128, 128), np.float32)
    msw[np.arange(128) ^ 1, np.arange(128)] = 1.0
    msw = msw.astype(BF16)
    idn = np.eye(128, dtype=np.float32).astype(BF16)

    # og_all row order is [pair-half hh? no: [half, core, head-in-half, dh]]:
    # AllGather of og_p[t][jp] rows [128] concatenates cores; gathered rows
    # n = jp*1024 + c*128 + hh*64 + d correspond to o = c*256 + (2*jp+hh)*64 + d.
    woT = np.asarray(wo).T                        # [o, d]
    perm = np.empty(D, dtype=np.int64)
    n = 0
    for jp in range(2):
        for c in range(N_CORES):
            for hh in range(2):
                o0 = c * 256 + (2 * jp + hh) * 64
                perm[n:n + 64] = np.arange(o0, o0 + 64)
                n += 64
    woT_p = woT[perm, :]

    in_maps = []
    for c in range(N_CORES):
        wq_c = np.asarray(wq)[OC * c: OC * (c + 1), :]        # [256, D]
        k_c = np.asarray(wk)[DH * c: DH * (c + 1), :]         # [64, D]
        v_c = np.asarray(wv)[DH * c: DH * (c + 1), :]
        wkv_c = np.concatenate([v_c, k_c], axis=0)            # [v, k]
        in_maps.append({
            "xT": xT,
            "wqT": np.ascontiguousarray(wq_c.T).astype(BF16),
            "wkvT": np.ascontiguousarray(wkv_c.T).astype(BF16),
            "woT": np.ascontiguousarray(woT_p[:, OC * c: OC * (c + 1)]).astype(BF16),
            "cosq": cosq, "sinq": sinq, "cosk": cosk, "sink": sink,
            "tri": tri, "mswap": msw, "ident": idn,
        })
    return in_maps


def kernel(x, pos, wq, wk, wv, wo):
    global LAST_RESULT
    if "nc" not in _CACHE:
        _CACHE["nc"] = _build_program()
    nc = _CACHE["nc"]
    in_maps = _host_prep(x, wq, wk, wv, wo, pos)
    res = run_bass_kernel_spmd(nc, in_maps, core_ids=list(range(N_CORES)))
    LAST_RESULT = res
    # y is emitted d-major ([256 d, 2048 s] per core); transpose on host
    y = np.concatenate([res.results[c]["y"].T for c in range(N_CORES)], axis=1)
    return y.reshape(1, S, D).astype(np.float32)


# revision 15
# speedup vs baseline: 1.1512x; 1.1512x over previous
"""Tensor-parallel GQA attention block for 8 Trainium2 NeuronCores.

Sharding: 32 q-heads / 8 kv-heads split across 8 cores (4 q-heads + 1
kv-head each).  Each core projects q/k/v from the full x, applies RoPE,
runs causal attention for its heads, AllGathers head outputs at
(s-tile, head-pair) granularity, and computes a distinct 256-column
slice of the final wo projection (emitted d-major; host transposes).

Attention inner loop processes q-heads in PAIRS: the two heads' score
matmuls use disjoint PE row groups (partition bases 0/64) so the
hardware runs them concurrently; their exp is one batched ACT call over
a 2-bank [128, 1024] PSUM tile; their attn@v matmuls share the vaug
stationary.  Score matmul for chunk kc+1 is emitted before attn@v of
chunk kc so the PE never drains while ACT computes exp.  The softmax
denominator reciprocal runs on the DVE (reciprocal_approx_fast), so the
ACT engine's spline tables never leave the Exp set.  The gathered
og_all row order is [head-pair-half, core, head, dh]; the host permutes
woT rows to match.  No fp32 matmuls anywhere (keeps FWL eligible).
"""

import sys

sys.path.insert(0, "/opt/trn_rl_repo")

import numpy as np
import ml_dtypes
from collections import deque
from contextlib import ExitStack

import concourse.bass as bass
import concourse.tile as tile
from concourse.tile_rust import add_dep_helper
from concourse import bacc, mybir
from concourse.bass import ds
from concourse.bass_utils import run_bass_kernel_spmd

BF16 = ml_dtypes.bfloat16
F32 = mybir.dt.float32
BF = mybir.dt.bfloat16

N_CORES = 8
S = 2048          # sequence length
D = 2048          # model dim
DH = 64           # head dim
HPC = 4           # q heads per core
THETA = 10000.0
ST = 512          # s-tile (free dim) size
NT = S // ST      # 4 s-tiles
DK = D // 128     # 16 contraction chunks
OC = HPC * DH     # 256 head-output columns per core

_CACHE = {}
LAST_RESULT = None


def _build_program():
    nc = bacc.Bacc("TRN2", target_bir_lowering=False, debug=False,
                   num_devices=N_CORES)

    def din(name, shape, dt):
        return nc.dram_tensor(name, shape, dt, kind="ExternalInput")

    xT_d = din("xT", [D, S], BF)
    wq_d = din("wqT", [D, OC], BF)
    wkv_d = din("wkvT", [D, 128], BF)      # columns: [v(64), k(64)]
    wo_d = din("woT", [D, OC], BF)         # rows o (pair-major), cols d-slice
    cosq_d = din("cosq", [128, S], BF)
    sinq_d = din("sinq", [128, S], BF)
    cosk_d = din("cosk", [128, S], BF)    # k tables live in rows 64..127
    sink_d = din("sink", [128, S], BF)
    tri_d = din("tri", [128, 128], BF)
    msw_d = din("mswap", [128, 128], BF)
    idn_d = din("ident", [128, 128], BF)

    y_d = nc.dram_tensor("y", [OC, S], F32, kind="ExternalOutput")  # d-major
    og_p = [[nc.dram_tensor(f"og{i}_{jp}", [128, ST], BF) for jp in range(2)]
            for i in range(NT)]
    oga_p = [[nc.dram_tensor(f"og_all{i}_{jp}", [N_CORES * 128, ST], BF,
                             addr_space="Shared") for jp in range(2)]
             for i in range(NT)]

    with tile.TileContext(nc) as tc:
        with ExitStack() as ctx:
            cp = ctx.enter_context(tc.tile_pool(name="const", bufs=1))
            psc = ctx.enter_context(tc.tile_pool(name="psc", bufs=2, space="PSUM"))
            pavp = ctx.enter_context(tc.tile_pool(name="pavp", bufs=2, space="PSUM"))
            psm = ctx.enter_context(tc.tile_pool(name="psm", bufs=2, space="PSUM"))
            rawp = ctx.enter_context(tc.tile_pool(name="raw", bufs=3))
            tmpp = ctx.enter_context(tc.tile_pool(name="tmp", bufs=3))
            expp = ctx.enter_context(tc.tile_pool(name="expp", bufs=4))
            nrmp = ctx.enter_context(tc.tile_pool(name="nrm", bufs=3))

            scr_d = nc.dram_tensor("scratch", [128, 8], F32)

            # ---- PE warm-up: enough to flip the HAM activity monitor to
            # full clock while the first input DMAs stream in.
            wsrc = cp.tile([128, ST], BF)
            nc.vector.memset(wsrc[:], 0.25)
            pw = psm.tile([128, ST], F32, tag="mm")
            NWARM = 12
            for i in range(NWARM):
                nc.tensor.matmul(pw[:], wsrc[:, 0:128], wsrc[:],
                                 start=(i == 0), stop=(i == NWARM - 1),
                                 skip_group_check=True)
            wout = rawp.tile([128, 8], F32, tag="wout")
            nc.vector.tensor_copy(wout[:], pw[:, 0:8])
            nc.gpsimd.dma_start(scr_d.ap(), wout[:])

            # ---- resident tensors (load order = consumption order) ----
            wkv_sb = cp.tile([128, DK, 128], BF)
            nc.sync.dma_start(wkv_sb[:], wkv_d.ap().rearrange("(ko p) m -> p ko m", p=128))
            bigs = [cp.tile([128, DK, ST], BF, name=f"big{i}", tag=f"big{i}") for i in range(NT)]
            for g in range(4):
                nc.sync.dma_start(
                    bigs[0][:, ds(4 * g, 4), :],
                    xT_d[ds(512 * g, 512), ds(0, ST)].rearrange("(ko p) s -> p ko s", p=128))
            wq_sb = cp.tile([128, DK, OC], BF)
            nc.sync.dma_start(wq_sb[:], wq_d.ap().rearrange("(ko p) m -> p ko m", p=128))
            # rope/mask tables go on the ACT hwdge queue so they are not
            # serialized behind the bulk x/weight loads on the sync queue
            cosk = cp.tile([128, S], BF); nc.scalar.dma_start(cosk[:], cosk_d.ap())
            sink = cp.tile([128, S], BF); nc.scalar.dma_start(sink[:], sink_d.ap())
            cosq = cp.tile([128, S], BF); nc.scalar.dma_start(cosq[:], cosq_d.ap())
            sinq = cp.tile([128, S], BF); nc.scalar.dma_start(sinq[:], sinq_d.ap())
            tri = cp.tile([128, 128], BF); nc.scalar.dma_start(tri[:], tri_d.ap())
            msw = cp.tile([128, 128], BF); nc.scalar.dma_start(msw[:], msw_d.ap())
            idn = cp.tile([128, 128], BF); nc.scalar.dma_start(idn[:], idn_d.ap())
            for t_ in (1, 2, 3):
                for g in range(4):
                    nc.sync.dma_start(
                        bigs[t_][:, ds(4 * g, 4), :],
                        xT_d[ds(512 * g, 512), ds(t_ * ST, ST)].rearrange("(ko p) s -> p ko s", p=128))
            wo_sb = cp.tile([128, DK, OC], BF)
            nc.sync.dma_start(wo_sb[:], wo_d.ap().rearrange("(ko p) m -> p ko m", p=128))

            qT = cp.tile([128, 2, S], BF)
            kT2 = cp.tile([128, S], BF)
            vaug = cp.tile([128, DK, DH + 1], BF)
            nc.vector.memset(vaug[:, :, DH:DH + 1], 1.0)

            # ---- phase 1 (projections + RoPE) as a list of small thunks
            # woven into the attention stream so dense PE work fills the
            # gaps of the ACT-paced attention pipeline.
            state = {}

            def mk_proj_thunks(t, j):
                X = bigs[t]

                def accum(d0, d1, first):
                    def th():
                        if first:
                            state[(t, j)] = psm.tile([128, ST], F32, tag="mm", name="projps")
                        ps = state[(t, j)]
                        for d in range(d0, d1):
                            lhsT = (wq_sb[:, d, ds(128 * j, 128)] if j < 2
                                    else wkv_sb[:, d, :])
                            nc.tensor.matmul(ps[:], lhsT, X[:, d, ds(0, ST)],
                                             start=(d == 0), stop=(d == DK - 1),
                                             skip_group_check=True)
                    return th

                def post():
                    ps = state.pop((t, j))
                    sl = ds(t * ST, ST)
                    raw = rawp.tile([128, ST], BF, tag="raw")
                    nc.vector.tensor_copy(raw[:], ps[:])
                    ps2 = psm.tile([128, ST], F32, tag="mm")
                    nc.tensor.matmul(ps2[:], msw[:], raw[:], start=True, stop=True)
                    if j < 2:
                        t1 = tmpp.tile([128, ST], F32, tag="tmp")
                        nc.vector.tensor_mul(t1[:], ps[:], cosq[:, sl])
                        t2 = tmpp.tile([128, ST], F32, tag="tmp")
                        nc.vector.tensor_mul(t2[:], ps2[:], sinq[:, sl])
                        nc.vector.tensor_add(qT[:, j, sl], t1[:], t2[:])
                    else:
                        t1 = tmpp.tile([128, ST], F32, tag="tmp")
                        nc.vector.tensor_mul(t1[64:128], ps[64:128], cosk[64:128, sl])
                        t2 = tmpp.tile([128, ST], F32, tag="tmp")
                        nc.vector.tensor_mul(t2[64:128], ps2[64:128], sink[64:128, sl])
                        nc.vector.tensor_add(kT2[64:128, sl], t1[64:128], t2[64:128])
                        # duplicate rotated k at partition base 0 (row-pack pair)
                        psd = psm.tile([64, ST], F32, tag="mm", name="psd")
                        nc.tensor.matmul(psd[:], idn[64:128, 64:128], kT2[64:128, sl],
                                         start=True, stop=True)
                        nc.vector.tensor_copy(kT2[0:64, sl], psd[:])
                        # v into [sk, dh] layout via PE transpose
                        for j4 in range(4):
                            pv = psm.tile([128, DH], BF, tag="mm", name="pv")
                            nc.tensor.transpose(pv[:], raw[0:64, ds(128 * j4, 128)],
                                                idn[0:64, 0:64])
                            nc.vector.tensor_copy(vaug[:, 4 * t + j4, 0:DH], pv[:])

                return [accum(0, 4, True), accum(4, 8, False),
                        accum(8, 12, False), accum(12, 16, False), post]

            # ---- phase 3: output projection, computed d-major ([d, s]) so
            # the wo stationaries stream 512-wide; host transposes at the
            # end.  Split at the half-contraction point so the first half
            # can run as soon as the pair-0 AllGather of that s-tile lands.
            p3state = {}

            def p3_mm(qt, dcol, o0, o1):
                X = bigs[qt]

                def th():
                    if o0 == 0:
                        p3state[(qt, dcol)] = psm.tile([128, ST], F32, tag="mm", name="p3py")
                    py = p3state[(qt, dcol)]
                    first = None
                    for oc in range(o0, o1):
                        mi = nc.tensor.matmul(py[:], wo_sb[:, oc, ds(128 * dcol, 128)],
                                              X[:, oc, :],
                                              start=(oc == 0), stop=(oc == DK - 1),
                                              skip_group_check=True)
                        if first is None:
                            first = mi
                    if o1 == DK:
                        py = p3state.pop((qt, dcol))
                        yts = tmpp.tile([128, ST], F32, tag="yts")
                        nc.scalar.copy(yts[:], py[:])
                        nc.scalar.dma_start(y_d[ds(128 * dcol, 128), ds(qt * ST, ST)],
                                            yts[:])
                    return first
                return th

            def mk_p3A(qt):
                return [p3_mm(qt, 0, 0, 4), p3_mm(qt, 0, 4, 8),
                        p3_mm(qt, 1, 0, 4), p3_mm(qt, 1, 4, 8)]

            def mk_p3B(qt):
                return [p3_mm(qt, 0, 8, 12), p3_mm(qt, 0, 12, 16),
                        p3_mm(qt, 1, 8, 12), p3_mm(qt, 1, 12, 16)]

            # ---- attention ----
            EXP = mybir.ActivationFunctionType.Exp

            def emit_norm(pav, t, jp, hh):
                # copy numerator+denominator out of PSUM right away (frees
                # the pav bank for the next pair), then normalize from SBUF.
                # the den row is copied to base 0 separately: custom-DVE ops
                # mis-handle partition-base offsets.
                oc = nrmp.tile([64, ST], F32, tag="oc")
                nc.vector.tensor_copy(oc[:], pav[0:DH, :])
                den = nrmp.tile([1, ST], F32, tag="den")
                nc.vector.tensor_copy(den[:], pav[DH:DH + 1, :])
                rec = nrmp.tile([1, ST], F32, tag="rec")
                nc.vector.reciprocal_approx_fast(rec[:], den[:])
                rep = nrmp.tile([64, ST], F32, tag="rep")
                nc.gpsimd.partition_broadcast(rep[:], rec[:])
                on = nrmp.tile([64, ST], BF, tag="on")
                nc.vector.tensor_mul(on[:], oc[:], rep[:])
                return nc.gpsimd.dma_start(og_p[t][jp][ds(DH * hh, DH), :], on[:])

            def emit_spair(t, jp, kc):
                sl = ds(t * ST, ST)
                ps = psc.tile([128, 1024], F32, tag="sc")
                nc.tensor.matmul(ps[:, 0:512], kT2[0:64, ds(128 * kc, 128)],
                                 qT[0:64, jp, sl], start=True, stop=True)
                nc.tensor.matmul(ps[:, 512:1024], kT2[64:128, ds(128 * kc, 128)],
                                 qT[64:128, jp, sl], start=True, stop=True)
                return ps

            # prologue: proj(t0) runs inline; kv first (longest dep chain
            # to the first score matmul).
            for th in (mk_proj_thunks(0, 2) + mk_proj_thunks(0, 0)
                       + mk_proj_thunks(0, 1)):
                th()

            wq = deque()
            for t in range(NT):
                if t + 1 < NT:
                    for j in (2, 0, 1):
                        wq.extend(mk_proj_thunks(t + 1, j))
                if t == 3:
                    for qt in range(2):
                        wq.extend(mk_p3A(qt))
                        wq.extend(mk_p3B(qt))
                    wq.extend(mk_p3A(2))
                nkc = 4 * t + 4
                nsteps = 2 * nkc
                # distribute woven work over the first ~3/4 of the block for
                # t<3 (so next tile's projections land before its block
                # starts); spread fully at t=3 (tail work arrives late).
                npace = nsteps if t == 3 else max(1, (3 * nsteps) // 4)
                thunks = list(wq)
                wq.clear()
                L = len(thunks)
                step = 0
                for jp in range(2):
                    pav0 = pavp.tile([128, ST], F32, tag="pav")
                    pav1 = pavp.tile([128, ST], F32, tag="pav")
                    ps_cur = emit_spair(t, jp, 0)
                    for kc in range(nkc):
                        ps_nxt = emit_spair(t, jp, kc + 1) if kc + 1 < nkc else None
                        # weave dense PE work into the exp wait
                        if step < npace:
                            lo = (step * L) // npace
                            hi = ((step + 1) * L) // npace
                            for th in thunks[lo:hi]:
                                th()
                        step += 1
                        et = expp.tile([128, 1024], BF, tag="exp")
                        nc.scalar.activation(et[:], ps_cur[:], EXP)
                        c = kc - 4 * t
                        if c >= 0:
                            for o in (0, 512):
                                if c > 0:
                                    nc.gpsimd.memset(et[:, o:o + 128 * c], 0.0)
                                nc.vector.tensor_mul(
                                    et[:, ds(o + 128 * c, 128)],
                                    et[:, ds(o + 128 * c, 128)], tri[:])
                        nc.tensor.matmul(pav0[0:DH + 1, :], vaug[:, kc, :],
                                         et[:, 0:512],
                                         start=(kc == 0), stop=(kc == nkc - 1),
                                         skip_group_check=True)
                        nc.tensor.matmul(pav1[0:DH + 1, :], vaug[:, kc, :],
                                         et[:, 512:1024],
                                         start=(kc == 0), stop=(kc == nkc - 1),
                                         skip_group_check=True)
                        ps_cur = ps_nxt
                    og_i0 = emit_norm(pav0, t, jp, 0)
                    og_i1 = emit_norm(pav1, t, jp, 1)
                    nc.gpsimd.collective_compute(
                        "AllGather", mybir.AluOpType.bypass,
                        replica_groups=[list(range(N_CORES))],
                        ins=[og_p[t][jp].ap()], outs=[oga_p[t][jp].ap()])
                    nc.sync.dma_start(
                        bigs[t][:, ds(8 * jp, 8), :],
                        oga_p[t][jp].ap().rearrange("(ko p) s -> p ko s", p=128))

            # endgame: held-back wo work, gated on the first og write of the
            # last pair so it executes DURING the final AllGather instead of
            # draining early, keeping the PE busy through the collective.
            gate_mm = None
            for th in mk_p3B(2) + mk_p3A(3) + mk_p3B(3):
                mi = th()
                if gate_mm is None and mi is not None:
                    gate_mm = mi
            add_dep_helper(og_i0.ins, gate_mm.ins, sync=True,
                           reason="hold endgame wo work until last og write")

    nc.compile()
    return nc


def _host_prep(x, wq, wk, wv, wo, pos):
    x2 = np.ascontiguousarray(np.asarray(x).reshape(S, D))
    xT = np.ascontiguousarray(x2.T).astype(BF16)

    posf = np.asarray(pos).astype(np.float32)
    fr = (1.0 / (np.float32(THETA) **
                 (np.arange(0, DH, 2, dtype=np.float32) / np.float32(DH))))
    pf = posf[:, None] * fr[None, :]              # [S, 32] f32
    cos = np.cos(pf).astype(np.float32)
    sin = np.sin(pf).astype(np.float32)
    pidx = np.arange(128)
    fi = (pidx % DH) // 2
    sign = np.where(pidx % 2 == 0, np.float32(-1.0), np.float32(1.0))
    cosq = np.ascontiguousarray(cos[:, fi].T)                  # [128, S]
    sinq = np.ascontiguousarray((sin[:, fi] * sign[None, :]).T)
    kscale = np.float32(1.0 / np.sqrt(DH))
    cosk = np.zeros((128, S), np.float32)
    sink = np.zeros((128, S), np.float32)
    cosk[64:128] = cosq[0:64] * kscale
    sink[64:128] = sinq[0:64] * kscale
    cosq = cosq.astype(BF16); sinq = sinq.astype(BF16)
    cosk = cosk.astype(BF16); sink = sink.astype(BF16)

    tri = np.triu(np.ones((128, 128), np.float32)).astype(BF16)
    msw = np.zeros((128, 128), np.float32)
    msw[np.arange(128) ^ 1, np.arange(128)] = 1.0
    msw = msw.astype(BF16)
    idn = np.eye(128, dtype=np.float32).astype(BF16)

    # og_all row order is [pair-half hh? no: [half, core, head-in-half, dh]]:
    # AllGather of og_p[t][jp] rows [128] concatenates cores; gathered rows
    # n = jp*1024 + c*128 + hh*64 + d correspond to o = c*256 + (2*jp+hh)*64 + d.
    woT = np.asarray(wo).T                        # [o, d]
    perm = np.empty(D, dtype=np.int64)
    n = 0
    for jp in range(2):
        for c in range(N_CORES):
            for hh in range(2):
                o0 = c * 256 + (2 * jp + hh) * 64
                perm[n:n + 64] = np.arange(o0, o0 + 64)
                n += 64
    woT_p = woT[perm, :]

    in_maps = []
    for c in range(N_CORES):
        wq_c = np.asarray(wq)[OC * c: OC * (c + 1), :]        # [256, D]
        k_c = np.asarray(wk)[DH * c: DH * (c + 1), :]         # [64, D]
        v_c = np.asarray(wv)[DH * c: DH * (c + 1), :]
        wkv_c = np.concatenate([v_c, k_c], axis=0)            # [v, k]
        in_maps.append({
            "xT": xT,
            "wqT": np.ascontiguousarray(wq_c.T).astype(BF16),
            "wkvT": np.ascontiguousarray(wkv_c.T).astype(BF16),
            "woT": np.ascontiguousarray(woT_p[:, OC * c: OC * (c + 1)]).astype(BF16),
            "cosq": cosq, "sinq": sinq, "cosk": cosk, "sink": sink,
            "tri": tri, "mswap": msw, "ident": idn,
        })
    return in_maps


def kernel(x, pos, wq, wk, wv, wo):
    global LAST_RESULT
    if "nc" not in _CACHE:
        _CACHE["nc"] = _build_program()
    nc = _CACHE["nc"]
    in_maps = _host_prep(x, wq, wk, wv, wo, pos)
    res = run_bass_kernel_spmd(nc, in_maps, core_ids=list(range(N_CORES)))
    LAST_RESULT = res
    # y is emitted d-major ([256 d, 2048 s] per core); transpose on host
    y = np.concatenate([res.results[c]["y"].T for c in range(N_CORES)], axis=1)
    return y.reshape(1, S, D).astype(np.float32)


# revision 16
# speedup vs baseline: 1.1651x; 1.0121x over previous
"""Tensor-parallel GQA attention block for 8 Trainium2 NeuronCores.

Sharding: 32 q-heads / 8 kv-heads split across 8 cores (4 q-heads + 1
kv-head each).  Each core projects q/k/v from the full x, applies RoPE,
runs causal attention for its heads, AllGathers head outputs at
(s-tile, head-pair) granularity, and computes a distinct 256-column
slice of the final wo projection (emitted d-major; host transposes).

Attention inner loop processes q-heads in PAIRS: the two heads' score
matmuls use disjoint PE row groups (partition bases 0/64) so the
hardware runs them concurrently; their exp is one batched ACT call over
a 2-bank [128, 1024] PSUM tile; their attn@v matmuls share the vaug
stationary.  Score matmul for chunk kc+1 is emitted before attn@v of
chunk kc so the PE never drains while ACT computes exp.  The softmax
denominator reciprocal runs on the DVE (reciprocal_approx_fast), so the
ACT engine's spline tables never leave the Exp set.  The gathered
og_all row order is [head-pair-half, core, head, dh]; the host permutes
woT rows to match.  No fp32 matmuls anywhere (keeps FWL eligible).
"""

import sys

sys.path.insert(0, "/opt/trn_rl_repo")

import numpy as np
import ml_dtypes
from collections import deque
from contextlib import ExitStack

import concourse.bass as bass
import concourse.tile as tile
from concourse import bacc, mybir
from concourse.bass import ds
from concourse.bass_utils import run_bass_kernel_spmd

BF16 = ml_dtypes.bfloat16
F32 = mybir.dt.float32
BF = mybir.dt.bfloat16

N_CORES = 8
S = 2048          # sequence length
D = 2048          # model dim
DH = 64           # head dim
HPC = 4           # q heads per core
THETA = 10000.0
ST = 512          # s-tile (free dim) size
NT = S // ST      # 4 s-tiles
DK = D // 128     # 16 contraction chunks
OC = HPC * DH     # 256 head-output columns per core

_CACHE = {}
LAST_RESULT = None


def _build_program():
    nc = bacc.Bacc("TRN2", target_bir_lowering=False, debug=False,
                   num_devices=N_CORES)

    def din(name, shape, dt):
        return nc.dram_tensor(name, shape, dt, kind="ExternalInput")

    xT_d = din("xT", [D, S], BF)
    wq_d = din("wqT", [D, OC], BF)
    wkv_d = din("wkvT", [D, 128], BF)      # columns: [v(64), k(64)]
    wo_d = din("woT", [D, OC], BF)         # rows o (pair-major), cols d-slice
    cosq_d = din("cosq", [128, S], BF)
    sinq_d = din("sinq", [128, S], BF)
    cosk_d = din("cosk", [128, S], BF)    # k tables live in rows 64..127
    sink_d = din("sink", [128, S], BF)
    tri_d = din("tri", [128, 128], BF)
    msw_d = din("mswap", [128, 128], BF)
    idn_d = din("ident", [128, 128], BF)

    y_d = nc.dram_tensor("y", [OC, S], F32, kind="ExternalOutput")  # d-major
    og_p = [[nc.dram_tensor(f"og{i}_{jp}", [128, ST], BF) for jp in range(2)]
            for i in range(NT)]
    oga_p = [[nc.dram_tensor(f"og_all{i}_{jp}", [N_CORES * 128, ST], BF,
                             addr_space="Shared") for jp in range(2)]
             for i in range(NT)]

    with tile.TileContext(nc) as tc:
        with ExitStack() as ctx:
            cp = ctx.enter_context(tc.tile_pool(name="const", bufs=1))
            psc = ctx.enter_context(tc.tile_pool(name="psc", bufs=2, space="PSUM"))
            pavp = ctx.enter_context(tc.tile_pool(name="pavp", bufs=2, space="PSUM"))
            psm = ctx.enter_context(tc.tile_pool(name="psm", bufs=2, space="PSUM"))
            rawp = ctx.enter_context(tc.tile_pool(name="raw", bufs=3))
            tmpp = ctx.enter_context(tc.tile_pool(name="tmp", bufs=3))
            expp = ctx.enter_context(tc.tile_pool(name="expp", bufs=4))
            nrmp = ctx.enter_context(tc.tile_pool(name="nrm", bufs=3))

            scr_d = nc.dram_tensor("scratch", [128, 8], F32)

            # ---- PE warm-up: enough to flip the HAM activity monitor to
            # full clock while the first input DMAs stream in.
            wsrc = cp.tile([128, ST], BF)
            nc.vector.memset(wsrc[:], 0.25)
            pw = psm.tile([128, ST], F32, tag="mm")
            NWARM = 12
            for i in range(NWARM):
                nc.tensor.matmul(pw[:], wsrc[:, 0:128], wsrc[:],
                                 start=(i == 0), stop=(i == NWARM - 1),
                                 skip_group_check=True)
            wout = rawp.tile([128, 8], F32, tag="wout")
            nc.vector.tensor_copy(wout[:], pw[:, 0:8])
            nc.gpsimd.dma_start(scr_d.ap(), wout[:])

            # ---- resident tensors (load order = consumption order) ----
            wkv_sb = cp.tile([128, DK, 128], BF)
            nc.sync.dma_start(wkv_sb[:], wkv_d.ap().rearrange("(ko p) m -> p ko m", p=128))
            bigs = [cp.tile([128, DK, ST], BF, name=f"big{i}", tag=f"big{i}") for i in range(NT)]
            for g in range(4):
                nc.sync.dma_start(
                    bigs[0][:, ds(4 * g, 4), :],
                    xT_d[ds(512 * g, 512), ds(0, ST)].rearrange("(ko p) s -> p ko s", p=128))
            wq_sb = cp.tile([128, DK, OC], BF)
            nc.sync.dma_start(wq_sb[:], wq_d.ap().rearrange("(ko p) m -> p ko m", p=128))
            # rope/mask tables go on the ACT hwdge queue so they are not
            # serialized behind the bulk x/weight loads on the sync queue
            cosk = cp.tile([128, S], BF); nc.scalar.dma_start(cosk[:], cosk_d.ap())
            sink = cp.tile([128, S], BF); nc.scalar.dma_start(sink[:], sink_d.ap())
            cosq = cp.tile([128, S], BF); nc.scalar.dma_start(cosq[:], cosq_d.ap())
            sinq = cp.tile([128, S], BF); nc.scalar.dma_start(sinq[:], sinq_d.ap())
            tri = cp.tile([128, 128], BF); nc.scalar.dma_start(tri[:], tri_d.ap())
            msw = cp.tile([128, 128], BF); nc.scalar.dma_start(msw[:], msw_d.ap())
            idn = cp.tile([128, 128], BF); nc.scalar.dma_start(idn[:], idn_d.ap())
            for t_ in (1, 2, 3):
                for g in range(4):
                    nc.sync.dma_start(
                        bigs[t_][:, ds(4 * g, 4), :],
                        xT_d[ds(512 * g, 512), ds(t_ * ST, ST)].rearrange("(ko p) s -> p ko s", p=128))
            wo_sb = cp.tile([128, DK, OC], BF)
            nc.sync.dma_start(wo_sb[:], wo_d.ap().rearrange("(ko p) m -> p ko m", p=128))

            qT = cp.tile([128, 2, S], BF)
            kT2 = cp.tile([128, S], BF)
            vaug = cp.tile([128, DK, DH + 1], BF)
            nc.vector.memset(vaug[:, :, DH:DH + 1], 1.0)

            # ---- phase 1 (projections + RoPE) as a list of small thunks
            # woven into the attention stream so dense PE work fills the
            # gaps of the ACT-paced attention pipeline.
            state = {}

            def mk_proj_thunks(t, j):
                X = bigs[t]

                def accum(d0, d1, first):
                    def th():
                        if first:
                            state[(t, j)] = psm.tile([128, ST], F32, tag="mm", name="projps")
                        ps = state[(t, j)]
                        for d in range(d0, d1):
                            lhsT = (wq_sb[:, d, ds(128 * j, 128)] if j < 2
                                    else wkv_sb[:, d, :])
                            nc.tensor.matmul(ps[:], lhsT, X[:, d, ds(0, ST)],
                                             start=(d == 0), stop=(d == DK - 1),
                                             skip_group_check=True)
                    return th

                def post():
                    ps = state.pop((t, j))
                    sl = ds(t * ST, ST)
                    raw = rawp.tile([128, ST], BF, tag="raw")
                    nc.vector.tensor_copy(raw[:], ps[:])
                    ps2 = psm.tile([128, ST], F32, tag="mm")
                    nc.tensor.matmul(ps2[:], msw[:], raw[:], start=True, stop=True)
                    if j < 2:
                        t1 = tmpp.tile([128, ST], F32, tag="tmp")
                        nc.vector.tensor_mul(t1[:], ps[:], cosq[:, sl])
                        t2 = tmpp.tile([128, ST], F32, tag="tmp")
                        nc.vector.tensor_mul(t2[:], ps2[:], sinq[:, sl])
                        nc.vector.tensor_add(qT[:, j, sl], t1[:], t2[:])
                    else:
                        t1 = tmpp.tile([128, ST], F32, tag="tmp")
                        nc.vector.tensor_mul(t1[64:128], ps[64:128], cosk[64:128, sl])
                        t2 = tmpp.tile([128, ST], F32, tag="tmp")
                        nc.vector.tensor_mul(t2[64:128], ps2[64:128], sink[64:128, sl])
                        nc.vector.tensor_add(kT2[64:128, sl], t1[64:128], t2[64:128])
                        # duplicate rotated k at partition base 0 (row-pack pair)
                        psd = psm.tile([64, ST], F32, tag="mm", name="psd")
                        nc.tensor.matmul(psd[:], idn[64:128, 64:128], kT2[64:128, sl],
                                         start=True, stop=True)
                        nc.vector.tensor_copy(kT2[0:64, sl], psd[:])
                        # v into [sk, dh] layout via PE transpose
                        for j4 in range(4):
                            pv = psm.tile([128, DH], BF, tag="mm", name="pv")
                            nc.tensor.transpose(pv[:], raw[0:64, ds(128 * j4, 128)],
                                                idn[0:64, 0:64])
                            nc.vector.tensor_copy(vaug[:, 4 * t + j4, 0:DH], pv[:])

                return [accum(0, 4, True), accum(4, 8, False),
                        accum(8, 12, False), accum(12, 16, False), post]

            # ---- phase 3: output projection, computed d-major ([d, s]) so
            # the wo stationaries stream 512-wide; host transposes at the
            # end.  Split at the half-contraction point so the first half
            # can run as soon as the pair-0 AllGather of that s-tile lands.
            p3state = {}

            def p3_mm(qt, dcol, o0, o1):
                X = bigs[qt]

                def th():
                    if o0 == 0:
                        p3state[(qt, dcol)] = psm.tile([128, ST], F32, tag="mm", name="p3py")
                    py = p3state[(qt, dcol)]
                    for oc in range(o0, o1):
                        nc.tensor.matmul(py[:], wo_sb[:, oc, ds(128 * dcol, 128)],
                                         X[:, oc, :],
                                         start=(oc == 0), stop=(oc == DK - 1),
                                         skip_group_check=True)
                    if o1 == DK:
                        py = p3state.pop((qt, dcol))
                        yts = tmpp.tile([128, ST], F32, tag="yts")
                        nc.vector.tensor_copy(yts[:], py[:])
                        nc.scalar.dma_start(y_d[ds(128 * dcol, 128), ds(qt * ST, ST)],
                                            yts[:])
                return th

            def mk_p3A(qt):
                return [p3_mm(qt, 0, 0, 4), p3_mm(qt, 0, 4, 8),
                        p3_mm(qt, 1, 0, 4), p3_mm(qt, 1, 4, 8)]

            def mk_p3B(qt):
                return [p3_mm(qt, 0, 8, 12), p3_mm(qt, 0, 12, 16),
                        p3_mm(qt, 1, 8, 12), p3_mm(qt, 1, 12, 16)]

            # ---- attention ----
            EXP = mybir.ActivationFunctionType.Exp

            def emit_norm(pav, t, jp, hh):
                # copy numerator+denominator out of PSUM right away (frees
                # the pav bank for the next pair), then normalize from SBUF.
                # the den row is copied to base 0 separately: custom-DVE ops
                # mis-handle partition-base offsets.
                oc = nrmp.tile([64, ST], F32, tag="oc")
                nc.vector.tensor_copy(oc[:], pav[0:DH, :])
                den = nrmp.tile([1, ST], F32, tag="den")
                nc.vector.tensor_copy(den[:], pav[DH:DH + 1, :])
                rec = nrmp.tile([1, ST], F32, tag="rec")
                nc.vector.reciprocal_approx_fast(rec[:], den[:])
                rep = nrmp.tile([64, ST], F32, tag="rep")
                nc.gpsimd.partition_broadcast(rep[:], rec[:])
                on = nrmp.tile([64, ST], BF, tag="on")
                nc.vector.tensor_mul(on[:], oc[:], rep[:])
                nc.gpsimd.dma_start(og_p[t][jp][ds(DH * hh, DH), :], on[:])

            def emit_spair(t, jp, kc):
                sl = ds(t * ST, ST)
                ps = psc.tile([128, 1024], F32, tag="sc")
                nc.tensor.matmul(ps[:, 0:512], kT2[0:64, ds(128 * kc, 128)],
                                 qT[0:64, jp, sl], start=True, stop=True)
                nc.tensor.matmul(ps[:, 512:1024], kT2[64:128, ds(128 * kc, 128)],
                                 qT[64:128, jp, sl], start=True, stop=True)
                return ps

            # prologue: proj(t0) runs inline; kv first (longest dep chain
            # to the first score matmul).
            for th in (mk_proj_thunks(0, 2) + mk_proj_thunks(0, 0)
                       + mk_proj_thunks(0, 1)):
                th()

            wq = deque()
            for t in range(NT):
                if t + 1 < NT:
                    for j in (2, 0, 1):
                        wq.extend(mk_proj_thunks(t + 1, j))
                if t == 3:
                    for qt in range(3):
                        wq.extend(mk_p3A(qt))
                        wq.extend(mk_p3B(qt))
                    wq.extend(mk_p3A(3))
                nkc = 4 * t + 4
                nsteps = 2 * nkc
                # distribute woven work over the first ~3/4 of the block for
                # t<3 (so next tile's projections land before its block
                # starts); spread fully at t=3 (tail work arrives late).
                npace = nsteps if t == 3 else max(1, (3 * nsteps) // 4)
                thunks = list(wq)
                wq.clear()
                L = len(thunks)
                step = 0
                for jp in range(2):
                    pav0 = pavp.tile([128, ST], F32, tag="pav")
                    pav1 = pavp.tile([128, ST], F32, tag="pav")
                    ps_cur = emit_spair(t, jp, 0)
                    for kc in range(nkc):
                        ps_nxt = emit_spair(t, jp, kc + 1) if kc + 1 < nkc else None
                        # weave dense PE work into the exp wait
                        if step < npace:
                            lo = (step * L) // npace
                            hi = ((step + 1) * L) // npace
                            for th in thunks[lo:hi]:
                                th()
                        step += 1
                        et = expp.tile([128, 1024], BF, tag="exp")
                        nc.scalar.activation(et[:], ps_cur[:], EXP)
                        c = kc - 4 * t
                        if c >= 0:
                            for o in (0, 512):
                                if c > 0:
                                    nc.gpsimd.memset(et[:, o:o + 128 * c], 0.0)
                                nc.vector.tensor_mul(
                                    et[:, ds(o + 128 * c, 128)],
                                    et[:, ds(o + 128 * c, 128)], tri[:])
                        nc.tensor.matmul(pav0[0:DH + 1, :], vaug[:, kc, :],
                                         et[:, 0:512],
                                         start=(kc == 0), stop=(kc == nkc - 1),
                                         skip_group_check=True)
                        nc.tensor.matmul(pav1[0:DH + 1, :], vaug[:, kc, :],
                                         et[:, 512:1024],
                                         start=(kc == 0), stop=(kc == nkc - 1),
                                         skip_group_check=True)
                        ps_cur = ps_nxt
                    emit_norm(pav0, t, jp, 0)
                    emit_norm(pav1, t, jp, 1)
                    nc.gpsimd.collective_compute(
                        "AllGather", mybir.AluOpType.bypass,
                        replica_groups=[list(range(N_CORES))],
                        ins=[og_p[t][jp].ap()], outs=[oga_p[t][jp].ap()])
                    nc.sync.dma_start(
                        bigs[t][:, ds(8 * jp, 8), :],
                        oga_p[t][jp].ap().rearrange("(ko p) s -> p ko s", p=128))

            # phase 3 tail: second half of the last s-tile
            for th in mk_p3B(3):
                th()

    nc.compile()
    return nc


def _host_prep(x, wq, wk, wv, wo, pos):
    x2 = np.ascontiguousarray(np.asarray(x).reshape(S, D))
    xT = np.ascontiguousarray(x2.T).astype(BF16)

    posf = np.asarray(pos).astype(np.float32)
    fr = (1.0 / (np.float32(THETA) **
                 (np.arange(0, DH, 2, dtype=np.float32) / np.float32(DH))))
    pf = posf[:, None] * fr[None, :]              # [S, 32] f32
    cos = np.cos(pf).astype(np.float32)
    sin = np.sin(pf).astype(np.float32)
    pidx = np.arange(128)
    fi = (pidx % DH) // 2
    sign = np.where(pidx % 2 == 0, np.float32(-1.0), np.float32(1.0))
    cosq = np.ascontiguousarray(cos[:, fi].T)                  # [128, S]
    sinq = np.ascontiguousarray((sin[:, fi] * sign[None, :]).T)
    kscale = np.float32(1.0 / np.sqrt(DH))
    cosk = np.zeros((128, S), np.float32)
    sink = np.zeros((128, S), np.float32)
    cosk[64:128] = cosq[0:64] * kscale
    sink[64:128] = sinq[0:64] * kscale
    cosq = cosq.astype(BF16); sinq = sinq.astype(BF16)
    cosk = cosk.astype(BF16); sink = sink.astype(BF16)

    tri = np.triu(np.ones((128, 128), np.float32)).astype(BF16)
    msw = np.zeros((128, 128), np.float32)
    msw[np.arange(128) ^ 1, np.arange(128)] = 1.0
    msw = msw.astype(BF16)
    idn = np.eye(128, dtype=np.float32).astype(BF16)

    # og_all row order is [pair-half hh? no: [half, core, head-in-half, dh]]:
    # AllGather of og_p[t][jp] rows [128] concatenates cores; gathered rows
    # n = jp*1024 + c*128 + hh*64 + d correspond to o = c*256 + (2*jp+hh)*64 + d.
    woT = np.asarray(wo).T                        # [o, d]
    perm = np.empty(D, dtype=np.int64)
    n = 0
    for jp in range(2):
        for c in range(N_CORES):
            for hh in range(2):
                o0 = c * 256 + (2 * jp + hh) * 64
                perm[n:n + 64] = np.arange(o0, o0 + 64)
                n += 64
    woT_p = woT[perm, :]

    in_maps = []
    for c in range(N_CORES):
        wq_c = np.asarray(wq)[OC * c: OC * (c + 1), :]        # [256, D]
        k_c = np.asarray(wk)[DH * c: DH * (c + 1), :]         # [64, D]
        v_c = np.asarray(wv)[DH * c: DH * (c + 1), :]
        wkv_c = np.concatenate([v_c, k_c], axis=0)            # [v, k]
        in_maps.append({
            "xT": xT,
            "wqT": np.ascontiguousarray(wq_c.T).astype(BF16),
            "wkvT": np.ascontiguousarray(wkv_c.T).astype(BF16),
            "woT": np.ascontiguousarray(woT_p[:, OC * c: OC * (c + 1)]).astype(BF16),
            "cosq": cosq, "sinq": sinq, "cosk": cosk, "sink": sink,
            "tri": tri, "mswap": msw, "ident": idn,
        })
    return in_maps


def kernel(x, pos, wq, wk, wv, wo):
    global LAST_RESULT
    if "nc" not in _CACHE:
        _CACHE["nc"] = _build_program()
    nc = _CACHE["nc"]
    in_maps = _host_prep(x, wq, wk, wv, wo, pos)
    res = run_bass_kernel_spmd(nc, in_maps, core_ids=list(range(N_CORES)))
    LAST_RESULT = res
    # y is emitted d-major ([256 d, 2048 s] per core); transpose on host
    y = np.concatenate([res.results[c]["y"].T for c in range(N_CORES)], axis=1)
    return y.reshape(1, S, D).astype(np.float32)
